# revision 10
# baseline (speedup 1.0000x reference)
"""Trainium2 Bass kernel for nn_Attn_VarLevel (sparse per-variable attention).

Math restructuring (exact, not approximate):
  reference:
    q  = queries @ Wq.T + bq                     [B,P,V,D]
    k  = keys @ Wkv.T + bkv                      [B,T,V,D]
    kc[b,p,v,n] = k[b, 32+p, c[b,v,n]]           (indices shared across p!)
    attn = softmax_n(q . kc / sqrt(D))
    out  = sum_n attn * kc
    y = concat(k[:, :32], out) @ Wout.T + bout

  kernel:
    * Full Gram GT[b,p,u,v] = <k_last[b,p,u], q[b,p,v]> via dense 64x64 matmuls
      (indices c only select entries of GT; duplicates handled by an integer
      multiplicity matrix mult[b][u,v] = #{n : c[b,v,n]==u}).
      softmax over n  ==  (mult * exp(s)) / sum_u(mult * exp(s))  exactly.
    * Since softmax weights sum to 1, the output projection folds into the
      keys:  y_attn = attnw @ (k_last @ Wout.T) + bout, and
      kp = k @ Wout.T + bout = keys @ Wfold + bfold with
      Wfold = Wkv.T @ Wout.T, bfold = Wout @ bkv + bout (host-precomputed).
      y[t<32] = kp[t<32] directly.
    * Z (softmax denominator) comes free as a 129th "ones" column of kp in the
      weighted-sum matmul; division is a per-partition tensor_scalar.

Sharding: data-parallel over batch, 2 batches per core on 8 cores.
"""

import sys

sys.path.insert(0, "/opt/trn_rl_repo")

import numpy as np

import concourse.bass as bass
import concourse.bacc as bacc
import concourse.mybir as mybir
import concourse.tile as tile
from concourse.bass_utils import run_bass_kernel_spmd
from concourse.masks import make_identity

B, P, T, V, N, D = 16, 96, 128, 64, 16, 128
NCORES = 8
BPC = B // NCORES          # batches per core
QTOK = P * V               # 6144 query tokens per batch
KTOK = T * V               # 8192 key tokens per batch
KTILES = KTOK // 128       # 64
QTILES = QTOK // 128       # 48
NCHUNK = 512               # matmul moving free dim
SCALE = float(D) ** -0.5

F32 = mybir.dt.float32

_cache = {}


def _build():
    if "nc" in _cache:
        return _cache["nc"]

    nc = bacc.Bacc(None, target_bir_lowering=False, debug=False)

    q_d = nc.declare_dram_parameter("queries", [BPC, QTOK, D], F32, isOutput=False)
    k_d = nc.declare_dram_parameter("keys", [BPC, KTOK, D], F32, isOutput=False)
    mult_d = nc.declare_dram_parameter("mult", [BPC, V, V], F32, isOutput=False)
    wq_d = nc.declare_dram_parameter("wq_t", [D, D], F32, isOutput=False)
    wkv_d = nc.declare_dram_parameter("wkv_t", [D, D], F32, isOutput=False)
    wfold_d = nc.declare_dram_parameter("wfold", [D, D], F32, isOutput=False)
    bq_d = nc.declare_dram_parameter("bq_col", [D, 1], F32, isOutput=False)
    bkv_d = nc.declare_dram_parameter("bkv_col", [D, 1], F32, isOutput=False)
    bfold_d = nc.declare_dram_parameter("bfold_rep", [D, D], F32, isOutput=False)
    out_d = nc.declare_dram_parameter("out", [BPC, KTOK, D], F32, isOutput=True)

    with tile.TileContext(nc) as tc:
        with (
            tc.tile_pool(name="const", bufs=1) as constp,
            tc.tile_pool(name="raw", bufs=6) as rawp,
            tc.tile_pool(name="chunkT", bufs=3) as chunkp,
            tc.tile_pool(name="perm", bufs=2) as permp,
            tc.tile_pool(name="at", bufs=4) as atp,
            tc.tile_pool(name="y", bufs=6) as yp,
            tc.tile_pool(name="rz", bufs=6) as rzp,
            tc.tile_pool(name="ps_t", bufs=2, space=bass.MemorySpace.PSUM) as ps_t,
            tc.tile_pool(name="ps_proj", bufs=2, space=bass.MemorySpace.PSUM) as ps_proj,
            tc.tile_pool(name="ps_g", bufs=2, space=bass.MemorySpace.PSUM) as ps_g,
            tc.tile_pool(name="ps_ws", bufs=2, space=bass.MemorySpace.PSUM) as ps_ws,
        ):
            ident = constp.tile([128, 128], F32, tag="ident")
            make_identity(nc, ident[:])
            wq_sb = constp.tile([D, D], F32, tag="wq")
            wkv_sb = constp.tile([D, D], F32, tag="wkv")
            wfold_sb = constp.tile([D, D], F32, tag="wfold")
            bq_sb = constp.tile([D, 1], F32, tag="bq")
            bkv_sb = constp.tile([D, 1], F32, tag="bkv")
            bfold_sb = constp.tile([D, D], F32, tag="bfold")
            nc.sync.dma_start(wq_sb[:], wq_d[:])
            nc.sync.dma_start(wkv_sb[:], wkv_d[:])
            nc.sync.dma_start(wfold_sb[:], wfold_d[:])
            nc.sync.dma_start(bq_sb[:], bq_d[:])
            nc.sync.dma_start(bkv_sb[:], bkv_d[:])
            nc.sync.dma_start(bfold_sb[:], bfold_d[:])

            for bi in range(BPC):
                # persistent per-batch tensors
                qT = permp.tile([D, QTOK], F32, tag="qT")       # projected queries^T
                kT = permp.tile([D, QTOK], F32, tag="kT")       # projected k_last^T
                kp = permp.tile([128, KTILES, D + 1], F32, tag="kp")  # folded keys
                mult_sb = permp.tile([2 * V, V], F32, tag="mult")
                nc.sync.dma_start(mult_sb[0:V, :], mult_d[bi])
                nc.sync.dma_start(mult_sb[V : 2 * V, :], mult_d[bi])
                # ones column for the Z (softmax denominator) trick
                nc.vector.memset(kp[:, :, D : D + 1], 1.0)

                # ---- keys side: transpose raw -> kp proj (+direct out for t<32)
                # ---- and kT proj for t>=32
                for c in range(KTOK // NCHUNK):          # 16 chunks of 512 tokens
                    ksT = chunkp.tile([128, NCHUNK], F32, tag="ksT")
                    for j in range(NCHUNK // 128):
                        ti = c * 4 + j
                        raw = rawp.tile([128, 128], F32, tag="raw")
                        nc.sync.dma_start(raw[:], k_d[bi, ti * 128 : (ti + 1) * 128, :])
                        pt = ps_t.tile([128, 128], F32, tag="pt")
                        nc.tensor.transpose(pt[:], raw[:], ident[:])
                        nc.vector.tensor_copy(ksT[:, j * 128 : (j + 1) * 128], pt[:])
                    # kp projection: one matmul per 128-token tile
                    for j in range(NCHUNK // 128):
                        ti = c * 4 + j
                        pp = ps_t.tile([128, 128], F32, tag="pt")
                        nc.tensor.matmul(
                            pp[:], ksT[:, j * 128 : (j + 1) * 128], wfold_sb[:],
                            start=True, stop=True,
                        )
                        if ti < 16:
                            # t < 32: output = kp + bfold directly
                            y = yp.tile([128, 128], F32, tag="y")
                            nc.vector.tensor_add(y[:], pp[:], bfold_sb[:])
                            nc.sync.dma_start(
                                out_d[bi, ti * 128 : (ti + 1) * 128, :], y[:]
                            )
                        else:
                            nc.scalar.copy(kp[:, ti, 0:D], pp[:])
                    if c >= 4:
                        # kT projection (Wkv) for score side, 512 tokens at once
                        pk = ps_proj.tile([128, NCHUNK], F32, tag="pproj")
                        nc.tensor.matmul(pk[:], wkv_sb[:], ksT[:], start=True, stop=True)
                        nc.scalar.activation(
                            kT[:, (c - 4) * NCHUNK : (c - 3) * NCHUNK], pk[:],
                            mybir.ActivationFunctionType.Identity,
                            bias=bkv_sb[:], scale=1.0,
                        )

                # ---- queries side: transpose raw -> qT proj
                for c in range(QTOK // NCHUNK):          # 12 chunks
                    qsT = chunkp.tile([128, NCHUNK], F32, tag="ksT")
                    for j in range(NCHUNK // 128):
                        ti = c * 4 + j
                        raw = rawp.tile([128, 128], F32, tag="raw")
                        nc.sync.dma_start(raw[:], q_d[bi, ti * 128 : (ti + 1) * 128, :])
                        pt = ps_t.tile([128, 128], F32, tag="pt")
                        nc.tensor.transpose(pt[:], raw[:], ident[:])
                        nc.vector.tensor_copy(qsT[:, j * 128 : (j + 1) * 128], pt[:])
                    pq = ps_proj.tile([128, NCHUNK], F32, tag="pproj")
                    nc.tensor.matmul(pq[:], wq_sb[:], qsT[:], start=True, stop=True)
                    nc.scalar.activation(
                        qT[:, c * NCHUNK : (c + 1) * NCHUNK], pq[:],
                        mybir.ActivationFunctionType.Identity,
                        bias=bq_sb[:], scale=1.0,
                    )

                # ---- attention: per twin (2 positions p share a 128-row tile)
                for tw in range(P // 2):
                    p0 = tw * 2
                    gps = ps_g.tile([128, 128], F32, tag="g")
                    # pair-even -> psum[0:64, 0:64]; pair-odd -> psum[64:128, 64:128]
                    nc.tensor.matmul(
                        gps[0:64, 0:64],
                        kT[:, p0 * 64 : (p0 + 1) * 64],
                        qT[:, p0 * 64 : (p0 + 1) * 64],
                        start=True, stop=True,
                    )
                    nc.tensor.matmul(
                        gps[64:128, 64:128],
                        kT[:, (p0 + 1) * 64 : (p0 + 2) * 64],
                        qT[:, (p0 + 1) * 64 : (p0 + 2) * 64],
                        start=True, stop=True, tile_position=(0, 64),
                    )
                    aT = atp.tile([128, 128], F32, tag="aT")
                    nc.scalar.activation(
                        aT[0:64, 0:64], gps[0:64, 0:64],
                        mybir.ActivationFunctionType.Exp, scale=SCALE,
                    )
                    nc.scalar.activation(
                        aT[64:128, 64:128], gps[64:128, 64:128],
                        mybir.ActivationFunctionType.Exp, scale=SCALE,
                    )
                    nc.vector.tensor_mul(
                        aT[0:64, 0:64], aT[0:64, 0:64], mult_sb[0:V, :]
                    )
                    nc.vector.tensor_mul(
                        aT[64:128, 64:128], aT[64:128, 64:128], mult_sb[V : 2 * V, :]
                    )
                    # weighted sum (+ Z in col 128): row-tiled pair of matmuls
                    ti0 = (32 + p0) // 2  # kp tile holding both positions
                    ws = ps_ws.tile([128, D + 1], F32, tag="ws")
                    nc.tensor.matmul(
                        ws[0:64, :], aT[0:64, 0:64], kp[0:64, ti0, :],
                        start=True, stop=True,
                    )
                    nc.tensor.matmul(
                        ws[64:128, :], aT[64:128, 64:128], kp[64:128, ti0, :],
                        start=True, stop=True, tile_position=(64, 64),
                    )
                    rz = rzp.tile([128, 1], F32, tag="rz")
                    nc.vector.reciprocal(rz[:], ws[:, D : D + 1])
                    y = yp.tile([128, 128], F32, tag="y")
                    nc.vector.tensor_scalar_mul(y[:], ws[:, 0:D], rz[:])
                    nc.gpsimd.tensor_add(y[:], y[:], bfold_sb[:])
                    tok0 = (32 + p0) * 64
                    nc.sync.dma_start(out_d[bi, tok0 : tok0 + 128, :], y[:])

    nc.finalize()
    _cache["nc"] = nc
    return nc


def prepare_in_maps(queries, keys, var_ccc, Wq, bq, Wkv, bkv, Wout, bout):
    queries = np.ascontiguousarray(np.asarray(queries, dtype=np.float32))
    keys = np.ascontiguousarray(np.asarray(keys, dtype=np.float32))
    var_ccc = np.asarray(var_ccc)
    Wq = np.asarray(Wq, dtype=np.float32)
    bq = np.asarray(bq, dtype=np.float32)
    Wkv = np.asarray(Wkv, dtype=np.float32)
    bkv = np.asarray(bkv, dtype=np.float32)
    Wout = np.asarray(Wout, dtype=np.float32)
    bout = np.asarray(bout, dtype=np.float32)

    # multiplicity matrices: mult[b][u, v] = #{n : var_ccc[b,v,n] == u}
    mult = np.zeros((B, V, V), dtype=np.float32)
    vv = np.repeat(np.arange(V), N)
    for b in range(B):
        np.add.at(mult[b], (var_ccc[b].reshape(-1).astype(np.int64), vv), 1.0)

    wq_t = np.ascontiguousarray(Wq.T)                    # lhsT for qT proj
    wkv_t = np.ascontiguousarray(Wkv.T)                  # lhsT for kT proj
    wfold = np.ascontiguousarray(Wkv.T @ Wout.T)         # keys -> kp
    bfold = Wout @ bkv + bout
    bfold_rep = np.ascontiguousarray(np.broadcast_to(bfold, (D, D)))
    bq_col = np.ascontiguousarray(bq.reshape(D, 1))
    bkv_col = np.ascontiguousarray(bkv.reshape(D, 1))

    in_maps = []
    for c in range(NCORES):
        sl = slice(c * BPC, (c + 1) * BPC)
        in_maps.append(
            {
                "queries": queries[sl].reshape(BPC, QTOK, D),
                "keys": keys[sl].reshape(BPC, KTOK, D),
                "mult": mult[sl],
                "wq_t": wq_t,
                "wkv_t": wkv_t,
                "wfold": wfold,
                "bq_col": bq_col,
                "bkv_col": bkv_col,
                "bfold_rep": bfold_rep,
            }
        )
    return in_maps


def assemble_out(res):
    return np.concatenate(
        [res.results[c]["out"].reshape(BPC, T, V, D) for c in range(NCORES)], axis=0
    )


def kernel(**inputs):
    nc = _build()
    in_maps = prepare_in_maps(**inputs)
    res = run_bass_kernel_spmd(nc, in_maps, list(range(NCORES)))
    return assemble_out(res)
